# revision 35
# baseline (speedup 1.0000x reference)
# Trainium2 Bass kernel for BasedCrossAttention (sparse_attention).
#
# Sharding: 8 cores = 2 batches x 4 head-groups (4 heads each).
# Per core: rmsnorm(x) -> q / window-kv projections, encoder -> kv
# projections, Taylor linear cross-attention (F=153 compressed feature
# map), sliding-window (64) causal self-attention, partial out-proj.
# Host: transposes inputs once, slices weights per core, sums the 4
# partial out-projections per batch and adds the residual.
#
# Layout: activations live feature-major [d, t]; attention outputs are
# computed d-major directly (kv_state / v_win as stationary operands)
# so no per-block PE transposes are needed. Per-token normalizers are
# broadcast across partitions on gpsimd off the PE critical path.
import math
from contextlib import ExitStack

import ml_dtypes
import numpy as np

import concourse.bass as bass
import concourse.tile as tile
from concourse import bacc, mybir
from concourse.bass_utils import run_bass_kernel_spmd

F32 = mybir.dt.float32
BF = mybir.dt.bfloat16
AF = mybir.ActivationFunctionType

D = 1024
NH = 16
HD = 64
FI = 16  # feature input dim
NQ = 136  # triu quadratic features
FTOT = NQ + FI + 1  # 153: [quad(136), lin(16), ones(1)]
C1 = FTOT - 128  # 25: second F chunk
WIN = 64
EPS_NORM = 1e-6
HPC = 4  # heads per core
DC = D // 128  # 8 d-model chunks


def build_program(T=2048, debug=False):
    NB = T // 128  # 128-token blocks
    NI4 = T // 512  # 512-token chunks
    assert T % 512 == 0

    nc = bacc.Bacc("TRN2", target_bir_lowering=False, debug=debug, num_devices=8)

    # ---- DRAM I/O ----
    xT = nc.dram_tensor("xT", [D, T], BF, kind="ExternalInput")
    encT = nc.dram_tensor("encT", [D, T], BF, kind="ExternalInput")
    Wq = nc.dram_tensor("Wq", [D, HPC * HD], BF, kind="ExternalInput")
    Wk = nc.dram_tensor("Wk", [D, HPC * HD], BF, kind="ExternalInput")
    Wv = nc.dram_tensor("Wv", [D, HPC * HD], BF, kind="ExternalInput")
    Wwk = nc.dram_tensor("Wwk", [D, HPC * HD], BF, kind="ExternalInput")
    Wwv = nc.dram_tensor("Wwv", [D, HPC * HD], BF, kind="ExternalInput")
    WqfA0 = nc.dram_tensor("WqfA0", [HD, 128], BF, kind="ExternalInput")
    WqfA1 = nc.dram_tensor("WqfA1", [HD, C1 - 1], BF, kind="ExternalInput")
    WqfB0 = nc.dram_tensor("WqfB0", [HD, 128], BF, kind="ExternalInput")
    WqfB1 = nc.dram_tensor("WqfB1", [HD, 8], BF, kind="ExternalInput")
    WkfA = nc.dram_tensor("WkfA", [HD, NQ + FI], BF, kind="ExternalInput")
    WkfB = nc.dram_tensor("WkfB", [HD, NQ], BF, kind="ExternalInput")
    WoutA = nc.dram_tensor("WoutA", [HPC * 128, D], BF, kind="ExternalInput")
    mask_pack_d = nc.dram_tensor("mask_pack", [128, 512], BF, kind="ExternalInput")
    ident_d = nc.dram_tensor("ident", [128, 128], BF, kind="ExternalInput")
    out_d = nc.dram_tensor("out", [D, T], F32, kind="ExternalOutput")

    with tile.TileContext(nc) as tc, ExitStack() as ctx:
        persist = ctx.enter_context(tc.tile_pool(name="persist", bufs=1))

        def load_w(dram, shape, rearr=None, eng=None):
            t = persist.tile(shape, BF, name=f"w_{dram.name}", tag=f"w_{dram.name}")
            src = dram.ap() if rearr is None else dram.ap().rearrange(rearr, p=128)
            (eng or nc.sync).dma_start(out=t, in_=src)
            return t

        def load_w2(dram, n, eng=None):
            # small [64, n] weight duplicated into both partition halves so it
            # can pair with operands at base_partition 0 or 64
            t = persist.tile([128, n], BF, name=f"w2_{dram.name}", tag=f"w2_{dram.name}")
            (eng or nc.sync).dma_start(out=t[0:64, :], in_=dram.ap())
            (eng or nc.sync).dma_start(out=t[64:128, :], in_=dram.ap())
            return t

        # ---- persistent SBUF ----
        ones_b = persist.tile([128, 1], BF)
        nc.vector.memset(ones_b, 1.0)
        eps_t = persist.tile([1, 1], F32)
        nc.vector.memset(eps_t, EPS_NORM)

        kvs = [persist.tile([128, 130], BF, tag=f"kvs{h}", name=f"kvs{h}") for h in range(HPC)]
        kT = [persist.tile([128, T], BF, tag=f"kT{hp}", name=f"kT{hp}") for hp in range(2)]
        kwT = [persist.tile([128, T], BF, tag=f"kwT{hp}", name=f"kwT{hp}") for hp in range(2)]
        qT = [persist.tile([128, T], BF, tag=f"qT{hp}", name=f"qT{hp}") for hp in range(2)]
        vA = persist.tile([128, HPC, NB, 65], BF, tag="vA", name="vA")
        vwa = persist.tile([128, HPC, NB, 65], BF, tag="vwa", name="vwa")
        nc.gpsimd.memset(vA, 1.0)
        nc.gpsimd.memset(vwa, 1.0)

        # ---- input prefetch: data first, weights interleaved, 2 queues ----
        encT_r = encT.ap().rearrange("(c p) t -> p c t", p=128)
        xT_r = xT.ap().rearrange("(c p) t -> p c t", p=128)
        inpool = ctx.enter_context(tc.tile_pool(name="inpool", bufs=1))
        ets, xts = {}, {}

        def fetch(i4):
            if i4 >= NI4:
                return
            et = inpool.tile([128, DC, 512], BF, tag="et", bufs=2, name=f"et{i4}")
            nc.sync.dma_start(out=et, in_=encT_r[:, :, i4 * 512:(i4 + 1) * 512])
            ets[i4] = et
            xt = inpool.tile([128, DC, 512], BF, tag="xt", bufs=2, name=f"xt{i4}")
            nc.scalar.dma_start(out=xt, in_=xT_r[:, :, i4 * 512:(i4 + 1) * 512])
            xts[i4] = xt

        wk = load_w(Wk, [128, DC, HPC * HD], "(c p) n -> p c n", eng=nc.sync)
        fetch(0)
        wv = load_w(Wv, [128, DC, HPC * HD], "(c p) n -> p c n", eng=nc.scalar)
        wkfA = load_w2(WkfA, NQ + FI, eng=nc.sync)
        wkfB = load_w2(WkfB, NQ, eng=nc.sync)
        wq = load_w(Wq, [128, DC, HPC * HD], "(c p) n -> p c n", eng=nc.scalar)
        fetch(1)
        wwk = load_w(Wwk, [128, DC, HPC * HD], "(c p) n -> p c n", eng=nc.scalar)
        wwv = load_w(Wwv, [128, DC, HPC * HD], "(c p) n -> p c n", eng=nc.scalar)
        wqfA0 = load_w2(WqfA0, 128, eng=nc.sync)
        wqfA1 = load_w2(WqfA1, C1 - 1, eng=nc.sync)
        wqfB0 = load_w2(WqfB0, 128, eng=nc.sync)
        wqfB1 = load_w2(WqfB1, 8, eng=nc.sync)
        mask_pack = load_w(mask_pack_d, [128, 512], eng=nc.scalar)
        ident = load_w(ident_d, [128, 128], eng=nc.scalar)
        wout = load_w(WoutA, [128, HPC, D], "(h p) n -> p h n", eng=nc.scalar)

        # =========== Phase AB: projections + kv_state, per 512 tokens ===========
        ctxAB = ExitStack()
        pAB = ctxAB.enter_context(tc.tile_pool(name="pAB", bufs=1))
        pABps = ctxAB.enter_context(tc.tile_pool(name="pABps", bufs=1, space="PSUM"))

        # kv_state accumulators (held across the whole phase)
        kvt = [pABps.tile([65, 320], F32, tag=f"kvt{hp}", bufs=1, name=f"kvt{hp}")
               for hp in range(2)]

        # warm the PE clock (HAM) while the first input DMAs land
        wsc = persist.tile([128, 512], BF, tag="wsc", name="wsc")
        nc.vector.memset(wsc, 0.125)
        for _ in range(36):
            wps = pABps.tile([128, 512], F32, tag="pj", bufs=2, name="warm")
            nc.tensor.matmul(wps, (wsc[:, 0:128]), (wsc), start=True, stop=True)

        for i4 in range(NI4):
            fetch(i4 + 2)
            tsl = slice(i4 * 512, (i4 + 1) * 512)
            et, xt = ets.pop(i4), xts.pop(i4)
            # --- B1: rmsnorm stats first; the rstd chain hides under kT/v ---
            ssp = pABps.tile([1, 512], F32, tag="ss", bufs=1)
            for c in range(DC):
                sq = pAB.tile([128, 512], BF, tag="sq", bufs=3)
                nc.scalar.square(sq, xt[:, c, :])
                nc.tensor.matmul(ssp, ones_b, sq, start=(c == 0), stop=(c == DC - 1))
            sd = pAB.tile([1, 512], F32, tag="sd", bufs=2)
            nc.scalar.activation(sd, ssp, AF.Sqrt, bias=eps_t[0:1, 0:1], scale=1.0 / D)
            rr = pAB.tile([1, 512], F32, tag="rr", bufs=2)
            nc.vector.reciprocal_approx_fast(rr, sd)
            rrb = pAB.tile([1, 512], BF, tag="rrb", bufs=2)
            nc.vector.tensor_copy(rrb, rr)
            rstdB = pAB.tile([128, 512], BF, tag="rstdB", bufs=2)
            nc.gpsimd.partition_broadcast(rstdB, rrb)
            # --- A1: kT = Wk^T enc (d-major) ---
            for hp in range(2):
                ps = pABps.tile([128, 512], F32, tag="pj", bufs=2)
                for c in range(DC):
                    nc.tensor.matmul(ps, (wk[:, c, hp * 128:(hp + 1) * 128]),
                                     (et[:, c, :]), start=(c == 0), stop=(c == DC - 1))
                nc.scalar.copy(kT[hp][:, tsl], ps)
            # --- A1: v (token-major, per 128-token block) ---
            for tb in range(4):
                blk = i4 * 4 + tb
                ps = pABps.tile([128, 512], F32, tag="pj", bufs=2)
                for c in range(DC):
                    nc.tensor.matmul(ps[:, 0:HPC * HD], (et[:, c, tb * 128:(tb + 1) * 128]),
                                     (wv[:, c, :]), start=(c == 0), stop=(c == DC - 1))
                if tb % 2 == 0:
                    nc.vector.tensor_copy(vA[:, :, blk, 0:HD], ps[:, 0:HPC * HD])
                else:
                    nc.scalar.copy(vA[:, :, blk, 0:HD], ps[:, 0:HPC * HD])
            # --- B1: xn = x * rstd ---
            xnt = pAB.tile([128, DC, 512], BF, tag="xnt", bufs=2)
            for c in range(DC):
                nc.vector.tensor_mul(xnt[:, c, :], xt[:, c, :], rstdB)
            # --- B1: q / kwin projections (d-major) ---
            for w_sb, dst in ((wq, qT), (wwk, kwT)):
                for hp in range(2):
                    ps = pABps.tile([128, 512], F32, tag="pj", bufs=2)
                    for c in range(DC):
                        nc.tensor.matmul(ps, (w_sb[:, c, hp * 128:(hp + 1) * 128]),
                                         (xnt[:, c, :]), start=(c == 0), stop=(c == DC - 1))
                    if hp == 0:
                        nc.scalar.copy(dst[hp][:, tsl], ps)
                    else:
                        nc.vector.tensor_copy(dst[hp][:, tsl], ps)
            # --- B1: vwin (token-major) ---
            for tb in range(4):
                blk = i4 * 4 + tb
                ps = pABps.tile([128, 512], F32, tag="pj", bufs=2)
                for c in range(DC):
                    nc.tensor.matmul(ps[:, 0:HPC * HD], (xnt[:, c, tb * 128:(tb + 1) * 128]),
                                     (wwv[:, c, :]), start=(c == 0), stop=(c == DC - 1))
                if tb % 2 == 0:
                    nc.vector.tensor_copy(vwa[:, :, blk, 0:HD], ps[:, 0:HPC * HD])
                else:
                    nc.scalar.copy(vwa[:, :, blk, 0:HD], ps[:, 0:HPC * HD])
            # --- A2 in three dense waves: reps MMs, phik builds, kvt MMs ---
            pairs = [(hp, tb) for hp in range(2) for tb in range(4)]
            repm = {}
            for hp, tb in pairs:
                tbs = slice(i4 * 512 + tb * 128, i4 * 512 + (tb + 1) * 128)
                reps = [pABps.tile([128, 288], F32, tag="rep", bufs=3, name=f"rep{u}")
                        for u in range(2)]
                for u in range(2):
                    ho = u * 64
                    nc.tensor.matmul(reps[u][:, 0:NQ + FI],
                                     (kT[hp][ho:ho + 64, tbs]), (wkfA[ho:ho + 64, :]))
                for u in range(2):
                    ho = u * 64
                    nc.tensor.matmul(reps[u][:, NQ + FI:NQ + FI + NQ],
                                     (kT[hp][ho:ho + 64, tbs]), (wkfB[ho:ho + 64, :]))
                phiks = []
                for u in range(2):
                    phik = pAB.tile([128, FTOT], BF, tag=f"phik{u}", bufs=8,
                                    name=f"phik{u}")
                    nc.vector.memset(phik[:, NQ + FI:FTOT], 1.0)
                    if u == 0:
                        nc.scalar.copy(phik[:, 0:NQ + FI], reps[u][:, 0:NQ + FI])
                    else:
                        nc.vector.tensor_copy(phik[:, 0:NQ + FI], reps[u][:, 0:NQ + FI])
                    nc.vector.tensor_mul(phik[:, 0:NQ], phik[:, 0:NQ],
                                         reps[u][:, NQ + FI:NQ + FI + NQ])
                    phiks.append(phik)
                repm[(hp, tb)] = phiks
            for hp, tb in pairs:
                gtb = i4 * 4 + tb
                phiks = repm[(hp, tb)]
                for u in range(2):
                    nc.tensor.matmul(kvt[hp][:, u * 160:u * 160 + FTOT],
                                     (vA[:, 2 * hp + u, gtb, :]), (phiks[u]),
                                     start=(gtb == 0), stop=(gtb == NB - 1))

        # --- kv_state -> F-major kvs[h] via PE transposes ---
        for hp in range(2):
            for u in range(2):
                h = 2 * hp + u
                kvt_sb = pAB.tile([65, FTOT], BF, tag="kvt_sb", bufs=2)
                nc.vector.tensor_copy(kvt_sb, kvt[hp][:, u * 160:u * 160 + FTOT])
                tp0 = pABps.tile([128, 512], F32, tag="pj", bufs=2, name="tp0").bitcast(BF)
                nc.tensor.transpose(tp0[0:128, 0:65], kvt_sb[:, 0:128], ident[0:65, 0:65])
                nc.vector.tensor_copy(kvs[h][:, 0:65], tp0[0:128, 0:65])
                tp1 = pABps.tile([128, 512], F32, tag="pj", bufs=2, name="tp1").bitcast(BF)
                nc.tensor.transpose(tp1[0:C1, 0:65], kvt_sb[:, 128:FTOT], ident[0:65, 0:65])
                nc.vector.tensor_copy(kvs[h][0:C1, 65:130], tp1[0:C1, 0:65])

        ctxAB.close()

        # =========== Phase C: features + attention + out-proj, per 512 q ===========
        with tc.tile_pool(name="pC", bufs=1) as pC, \
             tc.tile_pool(name="pCps", bufs=1, space="PSUM") as pCps:

            def emit_feat(c):
                csl = slice(c * 512, (c + 1) * 512)
                init = c < 2
                out = []
                for h in range(HPC):
                    hp, ho = h // 2, (h % 2) * 64
                    qtt = qT[hp][ho:ho + 64, csl]
                    p0 = pCps.tile([128, 512], F32, tag="A", bufs=2, name="p0")
                    pb0 = pCps.tile([128, 512], F32, tag="S", bufs=2, name="pb0")
                    p1b1 = pCps.tile([40, 512], F32, tag="S", bufs=2, name="p1b1")
                    nc.tensor.matmul(p0, (wqfA0[ho:ho + 64, :]), (qtt))
                    nc.tensor.matmul(pb0, (wqfB0[ho:ho + 64, :]), (qtt))
                    nc.tensor.matmul(p1b1[0:C1 - 1, :], (wqfA1[ho:ho + 64, :]), (qtt))
                    nc.tensor.matmul(p1b1[32:40, :], (wqfB1[ho:ho + 64, :]), (qtt),
                                     tile_position=(ho, 32))
                    pb_sb = pC.tile([128, 512], BF, tag="pb_sb", bufs=2)
                    if h % 2 == 0:
                        nc.scalar.copy(pb_sb, pb0)
                    else:
                        nc.vector.tensor_copy(pb_sb, pb0)
                    phiq0 = pC.tile([128, 512], BF, tag=f"phiq0_{h}", bufs=2,
                                    name=f"phiq0_{h}")
                    phiq1 = pC.tile([C1, 512], BF, tag=f"phiq1_{h}", bufs=2,
                                    name=f"phiq1_{h}")
                    nc.vector.tensor_mul(phiq0, p0, pb_sb)
                    if init:
                        nc.vector.memset(phiq1, 1.0)
                    nc.vector.tensor_copy(phiq1[0:C1 - 1, :], p1b1[0:C1 - 1, :])
                    nc.vector.tensor_mul(phiq1[0:8, :], phiq1[0:8, :], p1b1[32:40, :])
                    out.append((phiq0, phiq1))
                return out

            def emit_scores(c):
                exs = {}
                for sb in range(2):
                    j = 2 * c + sb
                    qsl = slice(j * 256, (j + 1) * 256)
                    qslA = slice(j * 256, j * 256 + 128)
                    qslB = slice(j * 256 + 128, (j + 1) * 256)
                    for hp in range(2):
                        # packed scores [kbL q0:128 | kb0 q0:256 | kb1 q128:256]
                        sps = [pCps.tile([128, 512], F32, tag="S", bufs=2,
                                         name=f"S{u}") for u in range(2)]
                        if j > 0:
                            for u in range(2):
                                ho = u * 64
                                nc.tensor.matmul(
                                    sps[u][:, 0:128],
                                    (kwT[hp][ho:ho + 64, (2 * j - 1) * 128:2 * j * 128]),
                                    (qT[hp][ho:ho + 64, qslA]))
                        for u in range(2):
                            ho = u * 64
                            nc.tensor.matmul(
                                sps[u][:, 128:384],
                                (kwT[hp][ho:ho + 64, 2 * j * 128:(2 * j + 1) * 128]),
                                (qT[hp][ho:ho + 64, qsl]))
                        for u in range(2):
                            ho = u * 64
                            nc.tensor.matmul(
                                sps[u][:, 384:512],
                                (kwT[hp][ho:ho + 64, (2 * j + 1) * 128:(2 * j + 2) * 128]),
                                (qT[hp][ho:ho + 64, qslB]))
                        for u in range(2):
                            ex = pC.tile([128, 512], BF, tag="ex", bufs=8,
                                         name=f"ex{u}")
                            eng = nc.vector if u == 0 else nc.gpsimd
                            if j > 0:
                                nc.scalar.activation(ex, sps[u], AF.Exp, scale=0.125)
                                eng.tensor_mul(ex, ex, mask_pack)
                            else:
                                nc.scalar.activation(ex[:, 128:512], sps[u][:, 128:512],
                                                     AF.Exp, scale=0.125)
                                eng.tensor_mul(ex[:, 128:512], ex[:, 128:512],
                                               mask_pack[:, 128:512])
                            exs[(sb, 2 * hp + u)] = ex
                return exs

            def emit_lw(c, phiqs, exs):
                combs = []
                for h in range(HPC):
                    phiq0, phiq1 = phiqs[h]
                    # linear path, d-major: [65, 512] (row 64 = normalizer)
                    lp = pCps.tile([65, 512], F32, tag="LW", bufs=4, name="lp")
                    nc.tensor.matmul(lp, (kvs[h][:, 0:65]), (phiq0),
                                     start=True, stop=False)
                    nc.tensor.matmul(lp, (kvs[h][0:C1, 65:130]), (phiq1),
                                     start=False, stop=True)
                    # window path, d-major: [65, 512] (row 64 = normalizer)
                    wp = pCps.tile([65, 512], F32, tag="LW", bufs=4, name="wp")
                    for sb in range(2):
                        j = 2 * c + sb
                        ex = exs[(sb, h)]
                        o = sb * 256
                        if j == 0:
                            nc.tensor.matmul(wp[:, 0:128], (vwa[:, h, 0, :]),
                                             (ex[:, 128:256]), start=True, stop=True)
                            nc.tensor.matmul(wp[:, 128:256], (vwa[:, h, 0, :]),
                                             (ex[:, 256:384]), start=True, stop=False)
                            nc.tensor.matmul(wp[:, 128:256], (vwa[:, h, 1, :]),
                                             (ex[:, 384:512]), start=False, stop=True)
                        else:
                            nc.tensor.matmul(wp[:, o:o + 256], (vwa[:, h, 2 * j, :]),
                                             (ex[:, 128:384]), start=True, stop=False,
                                             skip_group_check=True)
                            nc.tensor.matmul(wp[:, o:o + 128], (vwa[:, h, 2 * j - 1, :]),
                                             (ex[:, 0:128]), start=False, stop=True,
                                             skip_group_check=True)
                            nc.tensor.matmul(wp[:, o + 128:o + 256], (vwa[:, h, 2 * j + 1, :]),
                                             (ex[:, 384:512]), start=False, stop=True,
                                             skip_group_check=True)
                    # normalize: recip rows, broadcast, scale into comb (chan-major)
                    rlr = pC.tile([1, 512], F32, tag="rlr", bufs=2)
                    rwr = pC.tile([1, 512], F32, tag="rwr", bufs=2)
                    nc.scalar.copy(rlr, lp[64:65, :])
                    nc.scalar.copy(rwr, wp[64:65, :])
                    rl = pC.tile([1, 512], F32, tag="rl", bufs=2)
                    rw = pC.tile([1, 512], F32, tag="rw", bufs=2)
                    nc.vector.reciprocal_approx_fast(rl, rlr)
                    nc.vector.reciprocal_approx_fast(rw, rwr)
                    bl = pC.tile([64, 512], F32, tag="bl", bufs=2)
                    bw = pC.tile([64, 512], F32, tag="bw", bufs=2)
                    nc.gpsimd.partition_broadcast(bl, rl)
                    nc.gpsimd.partition_broadcast(bw, rw)
                    comb = pC.tile([128, 512], BF, tag=f"comb_{h}", bufs=2,
                                   name=f"comb_{h}")
                    nc.vector.tensor_mul(comb[0:64, :], lp[0:64, :], bl)
                    nc.vector.tensor_mul(comb[64:128, :], wp[0:64, :], bw)
                    combs.append(comb)
                return combs

            def emit_out(c, combs):
                gsl = slice(c * 512, (c + 1) * 512)
                for dc in range(DC):
                    po = pCps.tile([128, 512], F32, tag="A", bufs=2, name="po")
                    for h in range(HPC):
                        nc.tensor.matmul(po, (wout[:, h, dc * 128:(dc + 1) * 128]),
                                         (combs[h]), start=(h == 0), stop=(h == HPC - 1))
                    ob = pC.tile([128, 512], F32, tag="ob", bufs=2)
                    if dc % 2 == 0:
                        nc.scalar.copy(ob, po)
                        nc.sync.dma_start(out=out_d.ap()[dc * 128:(dc + 1) * 128, gsl],
                                          in_=ob)
                    else:
                        nc.vector.tensor_copy(ob, po)
                        nc.scalar.dma_start(out=out_d.ap()[dc * 128:(dc + 1) * 128, gsl],
                                            in_=ob)

            # software pipeline: feat/scores of chunk c+1 are emitted before
            # out-proj of chunk c so the PE never stalls on the normalize chain
            NCH = T // 512
            phiqs = emit_feat(0)
            exs = emit_scores(0)
            combs = emit_lw(0, phiqs, exs)
            for c in range(NCH):
                if c + 1 < NCH:
                    phiqs_n = emit_feat(c + 1)
                    exs_n = emit_scores(c + 1)
                else:
                    phiqs_n = exs_n = None
                emit_out(c, combs)
                if c + 1 < NCH:
                    combs = emit_lw(c + 1, phiqs_n, exs_n)
    nc.compile()
    return nc


# ---------------- host side ----------------

def _host_prep(x, encoder_out, norm_w, Wq, Wkv, Wqf, Wkf, Wwin, Wout, T):
    """Build the 8 per-core input maps."""
    nw = norm_w.astype(np.float64)
    WqF = (nw[:, None] * Wq).astype(np.float32)
    WwinF = (nw[:, None] * Wwin).astype(np.float32)
    Wk_all, Wv_all = Wkv[:, :D], Wkv[:, D:]
    Wwk_all, Wwv_all = WwinF[:, :D], WwinF[:, D:]

    ti, tj = np.triu_indices(FI)
    sc = np.where(ti == tj, 0.5, 2.0 ** -0.5).astype(np.float64)
    WqfA_f = (sc * Wqf[:, ti]).astype(np.float32)  # [64, 136]
    WqfB_f = Wqf[:, tj]
    WkfA_f = (sc * Wkf[:, ti]).astype(np.float32)
    WkfB_f = Wkf[:, tj]
    WqfA0 = WqfA_f[:, :128]
    WqfA1 = np.concatenate([WqfA_f[:, 128:], Wqf], axis=1)       # [64, 24]
    WqfB0 = WqfB_f[:, :128]
    WqfB1 = np.ascontiguousarray(WqfB_f[:, 128:])                # [64, 8]
    WkfA = np.concatenate([WkfA_f, Wkf], axis=1)                 # [64, 152]
    WkfB = WkfB_f                                                # [64, 136]

    kq, qq = np.arange(128)[:, None], np.arange(256)[None, :]
    mask_mid = ((kq <= qq) & (kq >= qq - WIN)).astype(np.float32)
    qq1 = np.arange(128)[None, :]
    mask_left = (kq >= qq1 + WIN).astype(np.float32)
    # packed S layout: [kbL q''0:128 | kb0 q''0:256 | kb1 q''128:256]
    mask_pack = np.concatenate([mask_left, mask_mid, mask_mid[:, 0:128]], axis=1)
    ident = np.eye(128, dtype=np.float32)

    in_maps = []
    for c in range(8):
        b, g = c // 4, c % 4
        cols = slice(g * HPC * HD, (g + 1) * HPC * HD)
        WoutA = np.empty((HPC * 128, D), np.float32)
        for h in range(HPC):
            hg = g * HPC + h
            WoutA[h * 128:h * 128 + 64] = Wout[hg * 64:(hg + 1) * 64]
            WoutA[h * 128 + 64:(h + 1) * 128] = Wout[D + hg * 64:D + (hg + 1) * 64]
        bf = ml_dtypes.bfloat16
        in_maps.append({
            "xT": np.ascontiguousarray(x[b, :T].T).astype(bf),
            "encT": np.ascontiguousarray(encoder_out[b, :T].T).astype(bf),
            "Wq": np.ascontiguousarray(WqF[:, cols]).astype(bf),
            "Wk": np.ascontiguousarray(Wk_all[:, cols]).astype(bf),
            "Wv": np.ascontiguousarray(Wv_all[:, cols]).astype(bf),
            "Wwk": np.ascontiguousarray(Wwk_all[:, cols]).astype(bf),
            "Wwv": np.ascontiguousarray(Wwv_all[:, cols]).astype(bf),
            "WqfA0": np.ascontiguousarray(WqfA0).astype(bf),
            "WqfA1": np.ascontiguousarray(WqfA1).astype(bf),
            "WqfB0": np.ascontiguousarray(WqfB0).astype(bf),
            "WqfB1": np.ascontiguousarray(WqfB1).astype(bf),
            "WkfA": np.ascontiguousarray(WkfA).astype(bf),
            "WkfB": np.ascontiguousarray(WkfB).astype(bf),
            "WoutA": WoutA.astype(bf),
            "mask_pack": mask_pack.astype(bf),
            "ident": ident.astype(bf),
        })
    return in_maps


_BUILD_CACHE = {}


def run_sharded(inputs, T=2048, trace=False):
    if T not in _BUILD_CACHE:
        _BUILD_CACHE[T] = build_program(T=T)
    nc = _BUILD_CACHE[T]
    in_maps = _host_prep(T=T, **inputs)
    res = run_bass_kernel_spmd(nc, in_maps, core_ids=list(range(8)), trace=trace)
    x = inputs["x"]
    out = np.array(x[:, :T], np.float32, copy=True)
    for c in range(8):
        out[c // 4] += res.results[c]["out"].T
    return out, res


def kernel(**inputs):
    inputs = {k: np.asarray(v, np.float32) for k, v in inputs.items()}
    out, _ = run_sharded(inputs, T=2048, trace=False)
    return out


# revision 36
# speedup vs baseline: 1.0227x; 1.0227x over previous
# Trainium2 Bass kernel for BasedCrossAttention (sparse_attention).
#
# Sharding: 8 cores = 2 batches x 4 head-groups (4 heads each).
# Per core: rmsnorm(x) -> q / window-kv projections, encoder -> kv
# projections, Taylor linear cross-attention (F=153 compressed feature
# map), sliding-window (64) causal self-attention, partial out-proj.
# Host: transposes inputs once, slices weights per core, sums the 4
# partial out-projections per batch and adds the residual.
#
# Layout: activations live feature-major [d, t]; attention outputs are
# computed d-major directly (kv_state / v_win as stationary operands)
# so no per-block PE transposes are needed. Per-token normalizers are
# broadcast across partitions on gpsimd off the PE critical path.
import math
from contextlib import ExitStack

import ml_dtypes
import numpy as np

import concourse.bass as bass
import concourse.tile as tile
from concourse import bacc, mybir
from concourse.bass_utils import run_bass_kernel_spmd

F32 = mybir.dt.float32
BF = mybir.dt.bfloat16
AF = mybir.ActivationFunctionType

D = 1024
NH = 16
HD = 64
FI = 16  # feature input dim
NQ = 136  # triu quadratic features
FTOT = NQ + FI + 1  # 153: [quad(136), lin(16), ones(1)]
C1 = FTOT - 128  # 25: second F chunk
WIN = 64
EPS_NORM = 1e-6
HPC = 4  # heads per core
DC = D // 128  # 8 d-model chunks


def build_program(T=2048, debug=False):
    NB = T // 128  # 128-token blocks
    NI4 = T // 512  # 512-token chunks
    assert T % 512 == 0

    nc = bacc.Bacc("TRN2", target_bir_lowering=False, debug=debug, num_devices=8)

    # ---- DRAM I/O ----
    xT = nc.dram_tensor("xT", [D, T], BF, kind="ExternalInput")
    encT = nc.dram_tensor("encT", [D, T], BF, kind="ExternalInput")
    Wq = nc.dram_tensor("Wq", [D, HPC * HD], BF, kind="ExternalInput")
    Wk = nc.dram_tensor("Wk", [D, HPC * HD], BF, kind="ExternalInput")
    Wv = nc.dram_tensor("Wv", [D, HPC * HD], BF, kind="ExternalInput")
    Wwk = nc.dram_tensor("Wwk", [D, HPC * HD], BF, kind="ExternalInput")
    Wwv = nc.dram_tensor("Wwv", [D, HPC * HD], BF, kind="ExternalInput")
    WqfA0 = nc.dram_tensor("WqfA0", [HD, 128], BF, kind="ExternalInput")
    WqfA1 = nc.dram_tensor("WqfA1", [HD, C1 - 1], BF, kind="ExternalInput")
    WqfB0 = nc.dram_tensor("WqfB0", [HD, 128], BF, kind="ExternalInput")
    WqfB1 = nc.dram_tensor("WqfB1", [HD, 8], BF, kind="ExternalInput")
    WkfA = nc.dram_tensor("WkfA", [HD, NQ + FI], BF, kind="ExternalInput")
    WkfB = nc.dram_tensor("WkfB", [HD, NQ], BF, kind="ExternalInput")
    WoutA = nc.dram_tensor("WoutA", [HPC * 128, D], BF, kind="ExternalInput")
    mask_pack_d = nc.dram_tensor("mask_pack", [128, 512], BF, kind="ExternalInput")
    ident_d = nc.dram_tensor("ident", [128, 128], BF, kind="ExternalInput")
    out_d = nc.dram_tensor("out", [D, T], F32, kind="ExternalOutput")

    with tile.TileContext(nc) as tc, ExitStack() as ctx:
        persist = ctx.enter_context(tc.tile_pool(name="persist", bufs=1))

        def load_w(dram, shape, rearr=None, eng=None):
            t = persist.tile(shape, BF, name=f"w_{dram.name}", tag=f"w_{dram.name}")
            src = dram.ap() if rearr is None else dram.ap().rearrange(rearr, p=128)
            (eng or nc.sync).dma_start(out=t, in_=src)
            return t

        def load_w2(dram, n, eng=None):
            # small [64, n] weight duplicated into both partition halves so it
            # can pair with operands at base_partition 0 or 64
            t = persist.tile([128, n], BF, name=f"w2_{dram.name}", tag=f"w2_{dram.name}")
            (eng or nc.sync).dma_start(out=t[0:64, :], in_=dram.ap())
            (eng or nc.sync).dma_start(out=t[64:128, :], in_=dram.ap())
            return t

        # ---- persistent SBUF ----
        ones_b = persist.tile([128, 1], BF)
        nc.vector.memset(ones_b, 1.0)
        eps_t = persist.tile([1, 1], F32)
        nc.vector.memset(eps_t, EPS_NORM)

        kvs = [persist.tile([128, 130], BF, tag=f"kvs{h}", name=f"kvs{h}") for h in range(HPC)]
        kT = [persist.tile([128, T], BF, tag=f"kT{hp}", name=f"kT{hp}") for hp in range(2)]
        kwT = [persist.tile([128, T], BF, tag=f"kwT{hp}", name=f"kwT{hp}") for hp in range(2)]
        qT = [persist.tile([128, T], BF, tag=f"qT{hp}", name=f"qT{hp}") for hp in range(2)]
        vA = persist.tile([128, HPC, NB, 65], BF, tag="vA", name="vA")
        vwa = persist.tile([128, HPC, NB, 65], BF, tag="vwa", name="vwa")
        nc.gpsimd.memset(vA, 1.0)
        nc.gpsimd.memset(vwa, 1.0)

        # ---- input prefetch: data first, weights interleaved, 2 queues ----
        encT_r = encT.ap().rearrange("(c p) t -> p c t", p=128)
        xT_r = xT.ap().rearrange("(c p) t -> p c t", p=128)
        inpool = ctx.enter_context(tc.tile_pool(name="inpool", bufs=1))
        ets, xts = {}, {}

        def fetch(i4):
            if i4 >= NI4:
                return
            et = inpool.tile([128, DC, 512], BF, tag="et", bufs=2, name=f"et{i4}")
            nc.sync.dma_start(out=et, in_=encT_r[:, :, i4 * 512:(i4 + 1) * 512])
            ets[i4] = et
            xt = inpool.tile([128, DC, 512], BF, tag="xt", bufs=2, name=f"xt{i4}")
            nc.scalar.dma_start(out=xt, in_=xT_r[:, :, i4 * 512:(i4 + 1) * 512])
            xts[i4] = xt

        wk = load_w(Wk, [128, DC, HPC * HD], "(c p) n -> p c n", eng=nc.sync)
        fetch(0)
        wv = load_w(Wv, [128, DC, HPC * HD], "(c p) n -> p c n", eng=nc.scalar)
        wkfA = load_w2(WkfA, NQ + FI, eng=nc.sync)
        wkfB = load_w2(WkfB, NQ, eng=nc.sync)
        wq = load_w(Wq, [128, DC, HPC * HD], "(c p) n -> p c n", eng=nc.scalar)
        fetch(1)
        wwk = load_w(Wwk, [128, DC, HPC * HD], "(c p) n -> p c n", eng=nc.scalar)
        wwv = load_w(Wwv, [128, DC, HPC * HD], "(c p) n -> p c n", eng=nc.scalar)
        wqfA0 = load_w2(WqfA0, 128, eng=nc.sync)
        wqfA1 = load_w2(WqfA1, C1 - 1, eng=nc.sync)
        wqfB0 = load_w2(WqfB0, 128, eng=nc.sync)
        wqfB1 = load_w2(WqfB1, 8, eng=nc.sync)
        mask_pack = load_w(mask_pack_d, [128, 512], eng=nc.scalar)
        ident = load_w(ident_d, [128, 128], eng=nc.scalar)
        wout = load_w(WoutA, [128, HPC, D], "(h p) n -> p h n", eng=nc.scalar)

        # =========== Phase AB: projections + kv_state, per 512 tokens ===========
        ctxAB = ExitStack()
        pAB = ctxAB.enter_context(tc.tile_pool(name="pAB", bufs=1))
        pABps = ctxAB.enter_context(tc.tile_pool(name="pABps", bufs=1, space="PSUM"))

        # kv_state accumulators (held across the whole phase)
        kvt = [pABps.tile([65, 320], F32, tag=f"kvt{hp}", bufs=1, name=f"kvt{hp}")
               for hp in range(2)]

        # warm the PE clock (HAM) while the first input DMAs land
        wsc = persist.tile([128, 512], BF, tag="wsc", name="wsc")
        nc.vector.memset(wsc, 0.125)
        for _ in range(44):
            wps = pABps.tile([128, 512], F32, tag="pj", bufs=2, name="warm")
            nc.tensor.matmul(wps, (wsc[:, 0:128]), (wsc), start=True, stop=True)

        for i4 in range(NI4):
            fetch(i4 + 2)
            tsl = slice(i4 * 512, (i4 + 1) * 512)
            et, xt = ets.pop(i4), xts.pop(i4)
            # --- B1: rmsnorm stats first; the rstd chain hides under kT/v ---
            ssp = pABps.tile([1, 512], F32, tag="ss", bufs=1)
            for c in range(DC):
                sq = pAB.tile([128, 512], BF, tag="sq", bufs=3)
                nc.scalar.square(sq, xt[:, c, :])
                nc.tensor.matmul(ssp, ones_b, sq, start=(c == 0), stop=(c == DC - 1))
            sd = pAB.tile([1, 512], F32, tag="sd", bufs=2)
            nc.scalar.activation(sd, ssp, AF.Sqrt, bias=eps_t[0:1, 0:1], scale=1.0 / D)
            rr = pAB.tile([1, 512], F32, tag="rr", bufs=2)
            nc.vector.reciprocal_approx_fast(rr, sd)
            rrb = pAB.tile([1, 512], BF, tag="rrb", bufs=2)
            nc.vector.tensor_copy(rrb, rr)
            rstdB = pAB.tile([128, 512], BF, tag="rstdB", bufs=2)
            nc.gpsimd.partition_broadcast(rstdB, rrb)
            # --- A1: kT = Wk^T enc (d-major) ---
            for hp in range(2):
                ps = pABps.tile([128, 512], F32, tag="pj", bufs=2)
                for c in range(DC):
                    nc.tensor.matmul(ps, (wk[:, c, hp * 128:(hp + 1) * 128]),
                                     (et[:, c, :]), start=(c == 0), stop=(c == DC - 1))
                nc.scalar.copy(kT[hp][:, tsl], ps)
            # --- A1: v (token-major, per 128-token block) ---
            for tb in range(4):
                blk = i4 * 4 + tb
                ps = pABps.tile([128, 512], F32, tag="pj", bufs=2)
                for c in range(DC):
                    nc.tensor.matmul(ps[:, 0:HPC * HD], (et[:, c, tb * 128:(tb + 1) * 128]),
                                     (wv[:, c, :]), start=(c == 0), stop=(c == DC - 1))
                if tb % 2 == 0:
                    nc.vector.tensor_copy(vA[:, :, blk, 0:HD], ps[:, 0:HPC * HD])
                else:
                    nc.scalar.copy(vA[:, :, blk, 0:HD], ps[:, 0:HPC * HD])
            # --- B1: xn = x * rstd ---
            xnt = pAB.tile([128, DC, 512], BF, tag="xnt", bufs=2)
            for c in range(DC):
                nc.vector.tensor_mul(xnt[:, c, :], xt[:, c, :], rstdB)
            # --- B1: q / kwin projections (d-major) ---
            for w_sb, dst in ((wq, qT), (wwk, kwT)):
                for hp in range(2):
                    ps = pABps.tile([128, 512], F32, tag="pj", bufs=2)
                    for c in range(DC):
                        nc.tensor.matmul(ps, (w_sb[:, c, hp * 128:(hp + 1) * 128]),
                                         (xnt[:, c, :]), start=(c == 0), stop=(c == DC - 1))
                    if hp == 0:
                        nc.scalar.copy(dst[hp][:, tsl], ps)
                    else:
                        nc.vector.tensor_copy(dst[hp][:, tsl], ps)
            # --- B1: vwin (token-major) ---
            for tb in range(4):
                blk = i4 * 4 + tb
                ps = pABps.tile([128, 512], F32, tag="pj", bufs=2)
                for c in range(DC):
                    nc.tensor.matmul(ps[:, 0:HPC * HD], (xnt[:, c, tb * 128:(tb + 1) * 128]),
                                     (wwv[:, c, :]), start=(c == 0), stop=(c == DC - 1))
                if tb % 2 == 0:
                    nc.vector.tensor_copy(vwa[:, :, blk, 0:HD], ps[:, 0:HPC * HD])
                else:
                    nc.scalar.copy(vwa[:, :, blk, 0:HD], ps[:, 0:HPC * HD])
            # --- A2 in three dense waves: reps MMs, phik builds, kvt MMs ---
            pairs = [(hp, tb) for hp in range(2) for tb in range(4)]
            repm = {}
            for hp, tb in pairs:
                tbs = slice(i4 * 512 + tb * 128, i4 * 512 + (tb + 1) * 128)
                reps = [pABps.tile([128, 288], F32, tag="rep", bufs=3, name=f"rep{u}")
                        for u in range(2)]
                for u in range(2):
                    ho = u * 64
                    nc.tensor.matmul(reps[u][:, 0:NQ + FI],
                                     (kT[hp][ho:ho + 64, tbs]), (wkfA[ho:ho + 64, :]))
                for u in range(2):
                    ho = u * 64
                    nc.tensor.matmul(reps[u][:, NQ + FI:NQ + FI + NQ],
                                     (kT[hp][ho:ho + 64, tbs]), (wkfB[ho:ho + 64, :]))
                phiks = []
                for u in range(2):
                    phik = pAB.tile([128, FTOT], BF, tag=f"phik{u}", bufs=8,
                                    name=f"phik{u}")
                    nc.vector.memset(phik[:, NQ + FI:FTOT], 1.0)
                    if u == 0:
                        nc.scalar.copy(phik[:, 0:NQ + FI], reps[u][:, 0:NQ + FI])
                    else:
                        nc.vector.tensor_copy(phik[:, 0:NQ + FI], reps[u][:, 0:NQ + FI])
                    nc.vector.tensor_mul(phik[:, 0:NQ], phik[:, 0:NQ],
                                         reps[u][:, NQ + FI:NQ + FI + NQ])
                    phiks.append(phik)
                repm[(hp, tb)] = phiks
            for hp, tb in pairs:
                gtb = i4 * 4 + tb
                phiks = repm[(hp, tb)]
                for u in range(2):
                    nc.tensor.matmul(kvt[hp][:, u * 160:u * 160 + FTOT],
                                     (vA[:, 2 * hp + u, gtb, :]), (phiks[u]),
                                     start=(gtb == 0), stop=(gtb == NB - 1))

        # --- kv_state -> F-major kvs[h] via PE transposes ---
        for hp in range(2):
            for u in range(2):
                h = 2 * hp + u
                kvt_sb = pAB.tile([65, FTOT], BF, tag="kvt_sb", bufs=2)
                nc.vector.tensor_copy(kvt_sb, kvt[hp][:, u * 160:u * 160 + FTOT])
                tp0 = pABps.tile([128, 512], F32, tag="pj", bufs=2, name="tp0").bitcast(BF)
                nc.tensor.transpose(tp0[0:128, 0:65], kvt_sb[:, 0:128], ident[0:65, 0:65])
                nc.vector.tensor_copy(kvs[h][:, 0:65], tp0[0:128, 0:65])
                tp1 = pABps.tile([128, 512], F32, tag="pj", bufs=2, name="tp1").bitcast(BF)
                nc.tensor.transpose(tp1[0:C1, 0:65], kvt_sb[:, 128:FTOT], ident[0:65, 0:65])
                nc.vector.tensor_copy(kvs[h][0:C1, 65:130], tp1[0:C1, 0:65])

        ctxAB.close()

        # =========== Phase C: features + attention + out-proj, per 512 q ===========
        with tc.tile_pool(name="pC", bufs=1) as pC, \
             tc.tile_pool(name="pCps", bufs=1, space="PSUM") as pCps:

            def emit_feat(c):
                csl = slice(c * 512, (c + 1) * 512)
                init = c < 2
                out = []
                for h in range(HPC):
                    hp, ho = h // 2, (h % 2) * 64
                    qtt = qT[hp][ho:ho + 64, csl]
                    p0 = pCps.tile([128, 512], F32, tag="A", bufs=2, name="p0")
                    pb0 = pCps.tile([128, 512], F32, tag="B", bufs=1, name="pb0")
                    p1b1 = pCps.tile([40, 512], F32, tag="S", bufs=2, name="p1b1")
                    nc.tensor.matmul(p0, (wqfA0[ho:ho + 64, :]), (qtt))
                    nc.tensor.matmul(pb0, (wqfB0[ho:ho + 64, :]), (qtt))
                    nc.tensor.matmul(p1b1[0:C1 - 1, :], (wqfA1[ho:ho + 64, :]), (qtt))
                    nc.tensor.matmul(p1b1[32:40, :], (wqfB1[ho:ho + 64, :]), (qtt),
                                     tile_position=(ho, 32))
                    pb_sb = pC.tile([128, 512], BF, tag="pb_sb", bufs=2)
                    if h % 2 == 0:
                        nc.scalar.copy(pb_sb, pb0)
                    else:
                        nc.vector.tensor_copy(pb_sb, pb0)
                    phiq0 = pC.tile([128, 512], BF, tag=f"phiq0_{h}", bufs=2,
                                    name=f"phiq0_{h}")
                    phiq1 = pC.tile([C1, 512], BF, tag=f"phiq1_{h}", bufs=2,
                                    name=f"phiq1_{h}")
                    nc.vector.tensor_mul(phiq0, p0, pb_sb)
                    if init:
                        nc.vector.memset(phiq1, 1.0)
                    nc.vector.tensor_copy(phiq1[0:C1 - 1, :], p1b1[0:C1 - 1, :])
                    nc.vector.tensor_mul(phiq1[0:8, :], phiq1[0:8, :], p1b1[32:40, :])
                    out.append((phiq0, phiq1))
                return out

            def emit_scores(c):
                exs = {}
                for sb in range(2):
                    j = 2 * c + sb
                    qsl = slice(j * 256, (j + 1) * 256)
                    qslA = slice(j * 256, j * 256 + 128)
                    qslB = slice(j * 256 + 128, (j + 1) * 256)
                    for hp in range(2):
                        # packed scores [kbL q0:128 | kb0 q0:256 | kb1 q128:256]
                        sps = [pCps.tile([128, 512], F32, tag="S", bufs=2,
                                         name=f"S{u}") for u in range(2)]
                        if j > 0:
                            for u in range(2):
                                ho = u * 64
                                nc.tensor.matmul(
                                    sps[u][:, 0:128],
                                    (kwT[hp][ho:ho + 64, (2 * j - 1) * 128:2 * j * 128]),
                                    (qT[hp][ho:ho + 64, qslA]))
                        for u in range(2):
                            ho = u * 64
                            nc.tensor.matmul(
                                sps[u][:, 128:384],
                                (kwT[hp][ho:ho + 64, 2 * j * 128:(2 * j + 1) * 128]),
                                (qT[hp][ho:ho + 64, qsl]))
                        for u in range(2):
                            ho = u * 64
                            nc.tensor.matmul(
                                sps[u][:, 384:512],
                                (kwT[hp][ho:ho + 64, (2 * j + 1) * 128:(2 * j + 2) * 128]),
                                (qT[hp][ho:ho + 64, qslB]))
                        for u in range(2):
                            ex = pC.tile([128, 512], BF, tag="ex", bufs=8,
                                         name=f"ex{u}")
                            eng = nc.vector if u == 0 else nc.gpsimd
                            if j > 0:
                                nc.scalar.activation(ex, sps[u], AF.Exp, scale=0.125)
                                eng.tensor_mul(ex, ex, mask_pack)
                            else:
                                nc.scalar.activation(ex[:, 128:512], sps[u][:, 128:512],
                                                     AF.Exp, scale=0.125)
                                eng.tensor_mul(ex[:, 128:512], ex[:, 128:512],
                                               mask_pack[:, 128:512])
                            exs[(sb, 2 * hp + u)] = ex
                return exs

            def emit_lw(c, phiqs, exs):
                combs = []
                for h in range(HPC):
                    phiq0, phiq1 = phiqs[h]
                    # linear path, d-major: [65, 512] (row 64 = normalizer)
                    lp = pCps.tile([65, 512], F32, tag="LW", bufs=3, name="lp")
                    nc.tensor.matmul(lp, (kvs[h][:, 0:65]), (phiq0),
                                     start=True, stop=False)
                    nc.tensor.matmul(lp, (kvs[h][0:C1, 65:130]), (phiq1),
                                     start=False, stop=True)
                    # window path, d-major: [65, 512] (row 64 = normalizer)
                    wp = pCps.tile([65, 512], F32, tag="LW", bufs=3, name="wp")
                    for sb in range(2):
                        j = 2 * c + sb
                        ex = exs[(sb, h)]
                        o = sb * 256
                        if j == 0:
                            nc.tensor.matmul(wp[:, 0:128], (vwa[:, h, 0, :]),
                                             (ex[:, 128:256]), start=True, stop=True)
                            nc.tensor.matmul(wp[:, 128:256], (vwa[:, h, 0, :]),
                                             (ex[:, 256:384]), start=True, stop=False)
                            nc.tensor.matmul(wp[:, 128:256], (vwa[:, h, 1, :]),
                                             (ex[:, 384:512]), start=False, stop=True)
                        else:
                            nc.tensor.matmul(wp[:, o:o + 256], (vwa[:, h, 2 * j, :]),
                                             (ex[:, 128:384]), start=True, stop=False,
                                             skip_group_check=True)
                            nc.tensor.matmul(wp[:, o:o + 128], (vwa[:, h, 2 * j - 1, :]),
                                             (ex[:, 0:128]), start=False, stop=True,
                                             skip_group_check=True)
                            nc.tensor.matmul(wp[:, o + 128:o + 256], (vwa[:, h, 2 * j + 1, :]),
                                             (ex[:, 384:512]), start=False, stop=True,
                                             skip_group_check=True)
                    # normalize: recip rows, broadcast, scale into comb (chan-major)
                    rlr = pC.tile([1, 512], F32, tag="rlr", bufs=2)
                    rwr = pC.tile([1, 512], F32, tag="rwr", bufs=2)
                    nc.scalar.copy(rlr, lp[64:65, :])
                    nc.scalar.copy(rwr, wp[64:65, :])
                    rl = pC.tile([1, 512], F32, tag="rl", bufs=2)
                    rw = pC.tile([1, 512], F32, tag="rw", bufs=2)
                    nc.vector.reciprocal_approx_fast(rl, rlr)
                    nc.vector.reciprocal_approx_fast(rw, rwr)
                    bl = pC.tile([64, 512], F32, tag="bl", bufs=2)
                    bw = pC.tile([64, 512], F32, tag="bw", bufs=2)
                    nc.gpsimd.partition_broadcast(bl, rl)
                    nc.gpsimd.partition_broadcast(bw, rw)
                    comb = pC.tile([128, 512], BF, tag=f"comb_{h}", bufs=2,
                                   name=f"comb_{h}")
                    nc.vector.tensor_mul(comb[0:64, :], lp[0:64, :], bl)
                    nc.vector.tensor_mul(comb[64:128, :], wp[0:64, :], bw)
                    combs.append(comb)
                return combs

            def emit_out(c, combs):
                gsl = slice(c * 512, (c + 1) * 512)
                for dc in range(DC):
                    po = pCps.tile([128, 512], F32, tag="A", bufs=2, name="po")
                    for h in range(HPC):
                        nc.tensor.matmul(po, (wout[:, h, dc * 128:(dc + 1) * 128]),
                                         (combs[h]), start=(h == 0), stop=(h == HPC - 1))
                    ob = pC.tile([128, 512], F32, tag="ob", bufs=2)
                    if dc % 2 == 0:
                        nc.scalar.copy(ob, po)
                        nc.sync.dma_start(out=out_d.ap()[dc * 128:(dc + 1) * 128, gsl],
                                          in_=ob)
                    else:
                        nc.vector.tensor_copy(ob, po)
                        nc.scalar.dma_start(out=out_d.ap()[dc * 128:(dc + 1) * 128, gsl],
                                            in_=ob)

            # software pipeline: feat/scores of chunk c+1 are emitted before
            # out-proj of chunk c so the PE never stalls on the normalize chain
            NCH = T // 512
            phiqs = emit_feat(0)
            exs = emit_scores(0)
            combs = emit_lw(0, phiqs, exs)
            for c in range(NCH):
                if c + 1 < NCH:
                    phiqs_n = emit_feat(c + 1)
                    exs_n = emit_scores(c + 1)
                else:
                    phiqs_n = exs_n = None
                emit_out(c, combs)
                if c + 1 < NCH:
                    combs = emit_lw(c + 1, phiqs_n, exs_n)
    nc.compile()
    return nc


# ---------------- host side ----------------

def _host_prep(x, encoder_out, norm_w, Wq, Wkv, Wqf, Wkf, Wwin, Wout, T):
    """Build the 8 per-core input maps."""
    nw = norm_w.astype(np.float64)
    WqF = (nw[:, None] * Wq).astype(np.float32)
    WwinF = (nw[:, None] * Wwin).astype(np.float32)
    Wk_all, Wv_all = Wkv[:, :D], Wkv[:, D:]
    Wwk_all, Wwv_all = WwinF[:, :D], WwinF[:, D:]

    ti, tj = np.triu_indices(FI)
    sc = np.where(ti == tj, 0.5, 2.0 ** -0.5).astype(np.float64)
    WqfA_f = (sc * Wqf[:, ti]).astype(np.float32)  # [64, 136]
    WqfB_f = Wqf[:, tj]
    WkfA_f = (sc * Wkf[:, ti]).astype(np.float32)
    WkfB_f = Wkf[:, tj]
    WqfA0 = WqfA_f[:, :128]
    WqfA1 = np.concatenate([WqfA_f[:, 128:], Wqf], axis=1)       # [64, 24]
    WqfB0 = WqfB_f[:, :128]
    WqfB1 = np.ascontiguousarray(WqfB_f[:, 128:])                # [64, 8]
    WkfA = np.concatenate([WkfA_f, Wkf], axis=1)                 # [64, 152]
    WkfB = WkfB_f                                                # [64, 136]

    kq, qq = np.arange(128)[:, None], np.arange(256)[None, :]
    mask_mid = ((kq <= qq) & (kq >= qq - WIN)).astype(np.float32)
    qq1 = np.arange(128)[None, :]
    mask_left = (kq >= qq1 + WIN).astype(np.float32)
    # packed S layout: [kbL q''0:128 | kb0 q''0:256 | kb1 q''128:256]
    mask_pack = np.concatenate([mask_left, mask_mid, mask_mid[:, 0:128]], axis=1)
    ident = np.eye(128, dtype=np.float32)

    in_maps = []
    for c in range(8):
        b, g = c // 4, c % 4
        cols = slice(g * HPC * HD, (g + 1) * HPC * HD)
        WoutA = np.empty((HPC * 128, D), np.float32)
        for h in range(HPC):
            hg = g * HPC + h
            WoutA[h * 128:h * 128 + 64] = Wout[hg * 64:(hg + 1) * 64]
            WoutA[h * 128 + 64:(h + 1) * 128] = Wout[D + hg * 64:D + (hg + 1) * 64]
        bf = ml_dtypes.bfloat16
        in_maps.append({
            "xT": np.ascontiguousarray(x[b, :T].T).astype(bf),
            "encT": np.ascontiguousarray(encoder_out[b, :T].T).astype(bf),
            "Wq": np.ascontiguousarray(WqF[:, cols]).astype(bf),
            "Wk": np.ascontiguousarray(Wk_all[:, cols]).astype(bf),
            "Wv": np.ascontiguousarray(Wv_all[:, cols]).astype(bf),
            "Wwk": np.ascontiguousarray(Wwk_all[:, cols]).astype(bf),
            "Wwv": np.ascontiguousarray(Wwv_all[:, cols]).astype(bf),
            "WqfA0": np.ascontiguousarray(WqfA0).astype(bf),
            "WqfA1": np.ascontiguousarray(WqfA1).astype(bf),
            "WqfB0": np.ascontiguousarray(WqfB0).astype(bf),
            "WqfB1": np.ascontiguousarray(WqfB1).astype(bf),
            "WkfA": np.ascontiguousarray(WkfA).astype(bf),
            "WkfB": np.ascontiguousarray(WkfB).astype(bf),
            "WoutA": WoutA.astype(bf),
            "mask_pack": mask_pack.astype(bf),
            "ident": ident.astype(bf),
        })
    return in_maps


_BUILD_CACHE = {}


def run_sharded(inputs, T=2048, trace=False):
    if T not in _BUILD_CACHE:
        _BUILD_CACHE[T] = build_program(T=T)
    nc = _BUILD_CACHE[T]
    in_maps = _host_prep(T=T, **inputs)
    res = run_bass_kernel_spmd(nc, in_maps, core_ids=list(range(8)), trace=trace)
    x = inputs["x"]
    out = np.array(x[:, :T], np.float32, copy=True)
    for c in range(8):
        out[c // 4] += res.results[c]["out"].T
    return out, res


def kernel(**inputs):
    inputs = {k: np.asarray(v, np.float32) for k, v in inputs.items()}
    out, _ = run_sharded(inputs, T=2048, trace=False)
    return out


# revision 37
# speedup vs baseline: 1.0472x; 1.0240x over previous
# Trainium2 Bass kernel for BasedCrossAttention (sparse_attention).
#
# Sharding: 8 cores = 2 batches x 4 head-groups (4 heads each).
# Each core computes, for its (batch, 4 heads):
#   rmsnorm(x) -> q / window-kv projections, encoder -> kv projections,
#   Taylor linear cross-attention (redundant F=273 feature map), sliding
#   window (64) causal self-attention, and a partial out-projection.
# Host: transposes inputs once, slices weights per core, sums the 4
# partial out-projections per batch and adds the residual.
#
# On-chip layout is "transposed" (feature-major): activations live as
# [d, t] with d on partitions so every matmul contracts over partitions.
import math
from contextlib import ExitStack

import ml_dtypes
import numpy as np

import concourse.bass as bass
import concourse.tile as tile
from concourse import bacc, mybir
from concourse.bass_utils import run_bass_kernel_spmd

F32 = mybir.dt.float32
BF = mybir.dt.bfloat16
AF = mybir.ActivationFunctionType

D = 1024
NH = 16
HD = 64
FI = 16  # feature input dim
NQ = 136  # triu quadratic features
FTOT = NQ + FI + 1  # 153: [quad(136), lin(16), ones(1)]
C1 = FTOT - 128  # 25: second F chunk
WIN = 64
EPS_NORM = 1e-6
EPS_DEN = 1e-6
HPC = 4  # heads per core
DC = D // 128  # 8 d-model chunks


def build_program(T=2048, debug=False):
    """One SPMD program; per-core variation comes only through inputs."""
    NB = T // 128  # 128-token blocks
    NI4 = T // 512  # 512-token chunks
    NSB = T // 256  # 256-token q superblocks
    TH = T // 2  # half for phiq/comb chunking
    CW = min(512, TH)  # free-dim chunk width within a half
    assert T % 512 == 0

    nc = bacc.Bacc("TRN2", target_bir_lowering=False, debug=debug, num_devices=8)

    # ---- DRAM I/O ----
    xT = nc.dram_tensor("xT", [D, T], BF, kind="ExternalInput")
    encT = nc.dram_tensor("encT", [D, T], BF, kind="ExternalInput")
    Wq = nc.dram_tensor("Wq", [D, HPC * HD], BF, kind="ExternalInput")
    Wk = nc.dram_tensor("Wk", [D, HPC * HD], BF, kind="ExternalInput")
    Wv = nc.dram_tensor("Wv", [D, HPC * HD], BF, kind="ExternalInput")
    Wwk = nc.dram_tensor("Wwk", [D, HPC * HD], BF, kind="ExternalInput")
    Wwv = nc.dram_tensor("Wwv", [D, HPC * HD], BF, kind="ExternalInput")
    WqfA0 = nc.dram_tensor("WqfA0", [HD, 128], BF, kind="ExternalInput")
    WqfA1 = nc.dram_tensor("WqfA1", [HD, C1 - 1], BF, kind="ExternalInput")
    WqfB0 = nc.dram_tensor("WqfB0", [HD, 128], BF, kind="ExternalInput")
    WqfB1 = nc.dram_tensor("WqfB1", [HD, 8], BF, kind="ExternalInput")
    WkfA = nc.dram_tensor("WkfA", [HD, NQ + FI], BF, kind="ExternalInput")
    WkfB = nc.dram_tensor("WkfB", [HD, NQ], BF, kind="ExternalInput")
    WoutA = nc.dram_tensor("WoutA", [HPC * 128, D], BF, kind="ExternalInput")
    mask_pack_d = nc.dram_tensor("mask_pack", [128, 512], BF, kind="ExternalInput")
    ident_d = nc.dram_tensor("ident", [128, 128], BF, kind="ExternalInput")
    out_d = nc.dram_tensor("out", [D, T], F32, kind="ExternalOutput")

    with tile.TileContext(nc) as tc, ExitStack() as ctx:
        persist = ctx.enter_context(tc.tile_pool(name="persist", bufs=1))

        def load_w(dram, shape, rearr=None, eng=None):
            t = persist.tile(shape, BF, name=f"w_{dram.name}", tag=f"w_{dram.name}")
            src = dram.ap() if rearr is None else dram.ap().rearrange(rearr, p=128)
            (eng or nc.sync).dma_start(out=t, in_=src)
            return t

        def load_w2(dram, n, eng=None):
            # small [64, n] weight duplicated into both partition halves so it
            # can pair with operands at base_partition 0 or 64
            t = persist.tile([128, n], BF, name=f"w2_{dram.name}", tag=f"w2_{dram.name}")
            (eng or nc.sync).dma_start(out=t[0:64, :], in_=dram.ap())
            (eng or nc.sync).dma_start(out=t[64:128, :], in_=dram.ap())
            return t

        wk = load_w(Wk, [128, DC, HPC * HD], "(c p) n -> p c n")
        wv = load_w(Wv, [128, DC, HPC * HD], "(c p) n -> p c n", eng=nc.scalar)
        wq = load_w(Wq, [128, DC, HPC * HD], "(c p) n -> p c n", eng=nc.scalar)
        wwk = load_w(Wwk, [128, DC, HPC * HD], "(c p) n -> p c n", eng=nc.scalar)
        wwv = load_w(Wwv, [128, DC, HPC * HD], "(c p) n -> p c n", eng=nc.scalar)
        mask_pack = load_w(mask_pack_d, [128, 512], eng=nc.scalar)
        ident = load_w(ident_d, [128, 128], eng=nc.scalar)

        ones_col = persist.tile([128, 1], F32)
        nc.vector.memset(ones_col, 1.0)
        ones_b = persist.tile([128, 1], BF)
        nc.vector.memset(ones_b, 1.0)
        eps_t = persist.tile([1, 1], F32)
        nc.vector.memset(eps_t, EPS_NORM)

        # Long-lived activations
        kvs = [persist.tile([128, 130], BF, tag=f"kvs{h}", name=f"kvs{h}") for h in range(HPC)]
        kwT = [persist.tile([128, T], BF, tag=f"kwT{hp}", name=f"kwT{hp}") for hp in range(2)]
        qT = [persist.tile([128, T], BF, tag=f"qT{hp}", name=f"qT{hp}") for hp in range(2)]
        vwa = persist.tile([128, HPC, NB, 65], BF, tag="vwa", name="vwa")
        rrT = persist.tile([128, NI4 * 4], F32, tag="rrT", name="rrT")  # rstd token-major

        # =============== Phase A: encoder side -> kv_state ===============
        with tc.tile_pool(name="pAkeep", bufs=1) as pAkeep:
            kT = [pAkeep.tile([128, T], BF, tag=f"kT{hp}", name=f"kT{hp}") for hp in range(2)]
            vA = pAkeep.tile([128, HPC, NB, 65], BF, tag="vA", name="vA")
            nc.gpsimd.memset(vA, 1.0)

            encT_r = encT.ap().rearrange("(c p) t -> p c t", p=128)
            ctxA1 = ExitStack()
            ctxA1.enter_context(nc.named_scope("A1_kv_proj"))
            pA = ctxA1.enter_context(tc.tile_pool(name="pA", bufs=2))
            pAps = ctxA1.enter_context(tc.tile_pool(name="pAps1", bufs=2, space="PSUM"))
            # warm the PE clock (HAM) while input DMAs land
            wsc = pA.tile([128, 512], BF, tag="wsc", bufs=1)
            nc.vector.memset(wsc, 0.125)
            wps = pAps.tile([128, 512], F32, tag="warm", bufs=1)
            for _ in range(38):
                nc.tensor.matmul(wps, (wsc[:, 0:128]), (wsc), start=True, stop=True)
            for i4 in range(NI4):
                et = pA.tile([128, DC, 512], BF, tag="enc", bufs=3)
                nc.sync.dma_start(out=et, in_=encT_r[:, :, i4 * 512:(i4 + 1) * 512])
                for hp in range(2):
                    ps = pAps.tile([128, 512], F32, tag="kt", bufs=3)
                    for c in range(DC):
                        nc.tensor.matmul(
                            ps, (wk[:, c, hp * 128:(hp + 1) * 128]),
                            (et[:, c, :]), start=(c == 0), stop=(c == DC - 1))
                    nc.scalar.copy(kT[hp][:, i4 * 512:(i4 + 1) * 512], ps)
                for tb in range(4):
                    blk = i4 * 4 + tb
                    ps = pAps.tile([128, HPC * HD], F32, tag="v")
                    for c in range(DC):
                        nc.tensor.matmul(
                            ps, (et[:, c, tb * 128:(tb + 1) * 128]),
                            (wv[:, c, :]), start=(c == 0), stop=(c == DC - 1))
                    # strided store into per-head 65-wide blocks (col 64 stays 1)
                    if blk % 2 == 0:
                        nc.vector.tensor_copy(vA[:, :, blk, 0:HD], ps)
                    else:
                        nc.scalar.copy(vA[:, :, blk, 0:HD], ps)


            ctxA1.close()
            with tc.tile_pool(name="pB1", bufs=2) as pB1, \
                 tc.tile_pool(name="pB1ps", bufs=2, space="PSUM") as pB1ps, \
                 nc.named_scope("B1_proj"):
                nc.gpsimd.memset(vwa, 1.0)
                xT_r = xT.ap().rearrange("(c p) t -> p c t", p=128)
                for i4 in range(NI4):
                    tsl = slice(i4 * 512, (i4 + 1) * 512)
                    xt = pB1.tile([128, DC, 512], BF, tag="xt", bufs=3)
                    nc.scalar.dma_start(out=xt, in_=xT_r[:, :, tsl])
                    # rmsnorm stats: sumsq over d via ones-matmul
                    ssp = pB1ps.tile([1, 512], F32, tag="ss", bufs=2)
                    for c in range(DC):
                        sq = pB1.tile([128, 512], BF, tag="sq")
                        nc.scalar.square(sq, xt[:, c, :])
                        nc.tensor.matmul(ssp, ones_b, sq,
                                         start=(c == 0), stop=(c == DC - 1))
                    sd = pB1.tile([1, 512], F32, tag="sd")
                    nc.scalar.activation(sd, ssp, AF.Sqrt, bias=eps_t[0:1, 0:1], scale=1.0 / D)
                    rr = pB1.tile([1, 512], F32, tag="rr")
                    rr_scr = pB1.tile([1, 512], F32, tag="rr_scr")
                    nc.vector.reciprocal_approx_accurate(rr, sd, rr_scr)
                    rstdB = pB1.tile([128, 512], F32, tag="rstdB")
                    nc.gpsimd.partition_broadcast(rstdB, rr)
                    # token-major rstd (for vwin scaling): transpose via matmul
                    for tb in range(4):
                        rtp = pB1ps.tile([128, 1], F32, tag="rt", bufs=1)
                        nc.tensor.matmul(rtp, rr[0:1, tb * 128:(tb + 1) * 128],
                                         ones_col[0:1, 0:1])
                        nc.vector.tensor_copy(rrT[:, i4 * 4 + tb:i4 * 4 + tb + 1], rtp)
                    # q / kwin projections (column-scaled by rstd)
                    for w_sb, dst in ((wq, qT), (wwk, kwT)):
                        for hp in range(2):
                            ps = pB1ps.tile([128, 512], F32, tag="qk", bufs=3)
                            for c in range(DC):
                                nc.tensor.matmul(
                                    ps, (w_sb[:, c, hp * 128:(hp + 1) * 128]),
                                    (xt[:, c, :]), start=(c == 0), stop=(c == DC - 1))
                            nc.vector.scalar_tensor_tensor(
                                dst[hp][:, tsl], ps, 1.0, rstdB,
                                op0=mybir.AluOpType.mult, op1=mybir.AluOpType.mult)
                    # vwin projection (row/token-scaled by rstd)
                    for tb in range(4):
                        blk = i4 * 4 + tb
                        ps = pB1ps.tile([128, HPC * HD], F32, tag="vw", bufs=2)
                        for c in range(DC):
                            nc.tensor.matmul(
                                ps, (xt[:, c, tb * 128:(tb + 1) * 128]),
                                (wwv[:, c, :]), start=(c == 0), stop=(c == DC - 1))
                        nc.scalar.activation(vwa[:, :, blk, 0:HD], ps, AF.Copy,
                                             scale=rrT[:, blk:blk + 1])


            # A2: features + kv_state^T per head (one wide matmul per block)
            wkfA = load_w2(WkfA, NQ + FI, eng=nc.scalar)
            wkfB = load_w2(WkfB, NQ, eng=nc.scalar)
            ctxA2 = ExitStack()
            ctxA2.enter_context(nc.named_scope("A2_kvstate"))
            pA2sb = ctxA2.enter_context(tc.tile_pool(name="pA2sb", bufs=1))
            pAps = ctxA2.enter_context(tc.tile_pool(name="pAps2", bufs=1, space="PSUM"))
            for hp in range(2):
                kvt2 = [pAps.tile([65, FTOT], F32, tag=f"kvt{u}", bufs=1,
                                  name=f"kvt{u}") for u in range(2)]
                for tb in range(NB):
                    ts_ = slice(tb * 128, (tb + 1) * 128)
                    reps = []
                    # paired K=64 matmuls: heads 2hp (rows 0:64), 2hp+1 (64:128)
                    for u in range(2):
                        ho = u * 64
                        repa = pAps.tile([128, NQ + FI], F32, tag=f"repa{u}",
                                         bufs=1, name=f"repa{u}")
                        nc.tensor.matmul(repa, (kT[hp][ho:ho + 64, ts_]),
                                         (wkfA[ho:ho + 64, :]))
                        reps.append(repa)
                    repbs = []
                    for u in range(2):
                        ho = u * 64
                        repb = pAps.tile([128, NQ], F32, tag=f"repb{u}",
                                         bufs=1, name=f"repb{u}")
                        nc.tensor.matmul(repb, (kT[hp][ho:ho + 64, ts_]),
                                         (wkfB[ho:ho + 64, :]))
                        repbs.append(repb)
                    phik2 = []
                    for u in range(2):
                        phik = pA2sb.tile([128, FTOT], BF, tag=f"phik{u}", bufs=2,
                                          name=f"phik{u}")
                        nc.vector.memset(phik, 1.0)  # ones col survives
                        if u == 0:
                            nc.scalar.copy(phik[:, 0:NQ + FI], reps[u])
                        else:
                            nc.vector.tensor_copy(phik[:, 0:NQ + FI], reps[u])
                        nc.vector.tensor_mul(phik[:, 0:NQ], phik[:, 0:NQ],
                                             repbs[u])
                        phik2.append(phik)
                    for u in range(2):
                        nc.tensor.matmul(kvt2[u], (vA[:, 2 * hp + u, tb, :]),
                                         (phik2[u]),
                                         start=(tb == 0), stop=(tb == NB - 1))
                for u in range(2):
                    h = 2 * hp + u
                    # kv_state^T [65, 273] -> F-major kvs[h] via PE transposes
                    kvt_sb = pA2sb.tile([65, FTOT], BF, tag="kvt_sb", bufs=2)
                    nc.vector.tensor_copy(kvt_sb, kvt2[u])
                    tp0 = pAps.tile([128, 65], BF, tag="tp", bufs=2)
                    nc.tensor.transpose(tp0, kvt_sb[:, 0:128], ident[0:65, 0:65])
                    nc.vector.tensor_copy(kvs[h][:, 0:65], tp0)
                    tp1 = pAps.tile([C1, 65], BF, tag="tp", bufs=2)
                    nc.tensor.transpose(tp1, kvt_sb[:, 128:FTOT], ident[0:65, 0:65])
                    nc.vector.tensor_copy(kvs[h][0:C1, 65:130], tp1)
            ctxA2.close()

        # B-feat + B-attn + B-out: one pool set across both halves
        wqfA0 = load_w2(WqfA0, 128, eng=nc.scalar)
        wqfA1 = load_w2(WqfA1, C1 - 1, eng=nc.scalar)
        wqfB0 = load_w2(WqfB0, 128, eng=nc.scalar)
        wqfB1 = load_w2(WqfB1, 8, eng=nc.scalar)
        wout = load_w(WoutA, [128, HPC, D], "(h p) n -> p h n", eng=nc.scalar)
        with tc.tile_pool(name="pB2", bufs=2) as pB2, \
             tc.tile_pool(name="pB2k", bufs=2) as pB2k, \
             tc.tile_pool(name="pB2ps", bufs=1, space="PSUM") as pB2ps:
            for half in range(2):
                hof = half * TH
                phiq0 = [pB2k.tile([128, TH], BF, tag=f"phiq0_{h}", name=f"phiq0_{h}")
                         for h in range(HPC)]
                phiq1 = [pB2k.tile([C1, TH], BF, tag=f"phiq1_{h}", name=f"phiq1_{h}")
                         for h in range(HPC)]
                combT = [pB2k.tile([128, TH], BF, tag=f"combT{h}", name=f"combT{h}")
                         for h in range(HPC)]

                # ---- features (phi_q, F-major) ----
                for h in range(HPC):
                    hp, ho = h // 2, (h % 2) * 64
                    nc.vector.memset(phiq1[h], 1.0)  # ones row survives
                    for j2 in range(TH // CW):
                        lsl = slice(j2 * CW, (j2 + 1) * CW)  # local
                        gsl = slice(hof + j2 * CW, hof + (j2 + 1) * CW)
                        qtt = qT[hp][ho:ho + 64, gsl]
                        p0 = pB2ps.tile([128, CW], F32, tag="big", bufs=4, name="p0")
                        p1 = pB2ps.tile([C1 - 1, CW], F32, tag="big", bufs=4, name="p1")
                        pb0 = pB2ps.tile([128, CW], F32, tag="big", bufs=4, name="pb0")
                        pb1 = pB2ps.tile([8, CW], F32, tag="big", bufs=4, name="pb1")
                        nc.tensor.matmul(p0, (wqfA0[ho:ho + 64, :]), (qtt))
                        nc.tensor.matmul(p1, (wqfA1[ho:ho + 64, :]), (qtt))
                        nc.tensor.matmul(pb0, (wqfB0[ho:ho + 64, :]), (qtt))
                        nc.tensor.matmul(pb1, (wqfB1[ho:ho + 64, :]), (qtt))
                        pb_sb = pB2.tile([128, CW], BF, tag="pb_sb", bufs=3)
                        if (h + j2) % 2 == 0:
                            nc.scalar.copy(pb_sb, pb0)
                        else:
                            nc.vector.tensor_copy(pb_sb, pb0)
                        nc.vector.tensor_mul(phiq0[h][:, lsl], p0, pb_sb)
                        nc.vector.tensor_copy(phiq1[h][0:C1 - 1, lsl], p1)
                        nc.vector.tensor_mul(phiq1[h][0:8, lsl], phiq1[h][0:8, lsl],
                                             pb1)

                # ---- attention ----
                for hp in range(2):
                    for jl in range(TH // 256):
                        j = half * (TH // 256) + jl  # global superblock
                        qsl = slice(j * 256, (j + 1) * 256)
                        qslA = slice(j * 256, j * 256 + 128)
                        qslB = slice(j * 256 + 128, (j + 1) * 256)
                        # packed scores [kbL q0:128 | kb0 q0:256 | kb1 q128:256],
                        # paired heads in disjoint PE row groups
                        sps = [pB2ps.tile([128, 512], F32, tag="big", bufs=4,
                                          name=f"S{u}") for u in range(2)]
                        if j > 0:
                            for u in range(2):
                                ho = u * 64
                                nc.tensor.matmul(
                                    sps[u][:, 0:128],
                                    (kwT[hp][ho:ho + 64, (2 * j - 1) * 128:2 * j * 128]),
                                    (qT[hp][ho:ho + 64, qslA]))
                        for u in range(2):
                            ho = u * 64
                            nc.tensor.matmul(
                                sps[u][:, 128:384],
                                (kwT[hp][ho:ho + 64, 2 * j * 128:(2 * j + 1) * 128]),
                                (qT[hp][ho:ho + 64, qsl]))
                        for u in range(2):
                            ho = u * 64
                            nc.tensor.matmul(
                                sps[u][:, 384:512],
                                (kwT[hp][ho:ho + 64, (2 * j + 1) * 128:(2 * j + 2) * 128]),
                                (qT[hp][ho:ho + 64, qslB]))
                        exs = []
                        for u in range(2):
                            ex = pB2.tile([128, 512], BF, tag=f"exps{u}", bufs=4,
                                          name=f"exps{u}")
                            eng = nc.vector if u == 0 else nc.gpsimd
                            if j > 0:
                                nc.scalar.activation(ex, sps[u], AF.Exp, scale=0.125)
                                eng.tensor_mul(ex, ex, mask_pack)
                            else:
                                nc.scalar.activation(ex[:, 128:512], sps[u][:, 128:512],
                                                     AF.Exp, scale=0.125)
                                eng.tensor_mul(ex[:, 128:512], ex[:, 128:512],
                                               mask_pack[:, 128:512])
                            exs.append(ex)
                        for u in range(2):
                            h = 2 * hp + u
                            ex = exs[u]
                            for qh in range(2):  # two 128-q blocks in superblock
                                qb = 2 * j + qh
                                lq = slice((qb * 128) - hof, (qb * 128) - hof + 128)
                                # linear path (cols 0:65) + window path (65:130)
                                lp = pB2ps.tile([128, 130], F32, tag="lin", bufs=3)
                                nc.tensor.matmul(lp[:, 0:65], (phiq0[h][:, lq]),
                                                 (kvs[h][:, 0:65]), start=True, stop=False)
                                nc.tensor.matmul(lp[:, 0:65], (phiq1[h][:, lq]),
                                                 (kvs[h][0:C1, 65:130]), start=False, stop=True)
                                if qh == 0:
                                    pvs = ([] if j == 0 else [slice(0, 128)]) + [slice(128, 256)]
                                else:
                                    pvs = [slice(256, 384), slice(384, 512)]
                                kb0 = qb - 1 if (qh == 0 and j > 0) or qh == 1 else qb
                                for ki, exsl in enumerate(pvs):
                                    kb = kb0 + ki
                                    nc.tensor.matmul(lp[:, 65:130], (ex[:, exsl]),
                                                     (vwa[:, h, kb, :]),
                                                     start=(ki == 0), stop=(ki == len(pvs) - 1))
                                # one reciprocal for both normalizers (cols 64, 129)
                                rnl = pB2.tile([128, 2], F32, tag="rnl", bufs=4)
                                nc.vector.reciprocal(rnl, lp[:, 64:130:65])
                                comb = pB2.tile([128, 128], BF, tag="comb", bufs=4)
                                nc.vector.tensor_scalar_mul(comb[:, 0:64], lp[:, 0:64],
                                                            rnl[:, 0:1])
                                nc.vector.tensor_scalar_mul(comb[:, 64:128], lp[:, 65:129],
                                                            rnl[:, 1:2])
                                # transpose [tok, ch] -> [ch, tok]
                                ct = pB2ps.tile([128, 128], BF, tag="ct", bufs=1)
                                nc.tensor.transpose(ct, comb, ident)
                                if (u + qh) % 2 == 0:
                                    nc.scalar.copy(combT[h][:, lq], ct)
                                else:
                                    nc.vector.tensor_copy(combT[h][:, lq], ct)

                # ---- out-projection (partial over this core's heads) ----
                for j2 in range(TH // CW):
                    lsl = slice(j2 * CW, (j2 + 1) * CW)
                    gsl = slice(hof + j2 * CW, hof + (j2 + 1) * CW)
                    for dc in range(DC):
                        po = pB2ps.tile([128, CW], F32, tag="big", bufs=4, name="po")
                        for h in range(HPC):
                            nc.tensor.matmul(
                                po, (wout[:, h, dc * 128:(dc + 1) * 128]),
                                (combT[h][:, lsl]),
                                start=(h == 0), stop=(h == HPC - 1))
                        ob = pB2.tile([128, CW], F32, tag="ob")
                        nc.scalar.copy(ob, po)
                        nc.sync.dma_start(out=out_d.ap()[dc * 128:(dc + 1) * 128, gsl],
                                          in_=ob)
    nc.compile()
    return nc


# ---------------- host side ----------------

def _host_prep(x, encoder_out, norm_w, Wq, Wkv, Wqf, Wkf, Wwin, Wout, T):
    """Build the 8 per-core input maps."""
    nw = norm_w.astype(np.float64)
    WqF = (nw[:, None] * Wq).astype(np.float32)
    WwinF = (nw[:, None] * Wwin).astype(np.float32)
    Wk_all, Wv_all = Wkv[:, :D], Wkv[:, D:]
    Wwk_all, Wwv_all = WwinF[:, :D], WwinF[:, D:]

    ti, tj = np.triu_indices(FI)
    sc = np.where(ti == tj, 0.5, 2.0 ** -0.5).astype(np.float64)
    WqfA_f = (sc * Wqf[:, ti]).astype(np.float32)  # [64, 136]
    WqfB_f = Wqf[:, tj]
    WkfA_f = (sc * Wkf[:, ti]).astype(np.float32)
    WkfB_f = Wkf[:, tj]
    WqfA0 = WqfA_f[:, :128]
    WqfA1 = np.concatenate([WqfA_f[:, 128:], Wqf], axis=1)       # [64, 24]
    WqfB0 = WqfB_f[:, :128]
    WqfB1 = np.ascontiguousarray(WqfB_f[:, 128:])                # [64, 8]
    WkfA = np.concatenate([WkfA_f, Wkf], axis=1)                 # [64, 152]
    WkfB = WkfB_f                                                # [64, 136]

    kq, qq = np.arange(128)[:, None], np.arange(256)[None, :]
    mask_mid = ((kq <= qq) & (kq >= qq - WIN)).astype(np.float32)
    qq1 = np.arange(128)[None, :]
    mask_left = (kq >= qq1 + WIN).astype(np.float32)
    # packed S layout: [kbL q''0:128 | kb0 q''0:256 | kb1 q''128:256]
    mask_pack = np.concatenate([mask_left, mask_mid, mask_mid[:, 0:128]], axis=1)
    ident = np.eye(128, dtype=np.float32)

    in_maps = []
    for c in range(8):
        b, g = c // 4, c % 4
        cols = slice(g * HPC * HD, (g + 1) * HPC * HD)
        WoutA = np.empty((HPC * 128, D), np.float32)
        for h in range(HPC):
            hg = g * HPC + h
            WoutA[h * 128:h * 128 + 64] = Wout[hg * 64:(hg + 1) * 64]
            WoutA[h * 128 + 64:(h + 1) * 128] = Wout[D + hg * 64:D + (hg + 1) * 64]
        bf = ml_dtypes.bfloat16
        in_maps.append({
            "xT": np.ascontiguousarray(x[b, :T].T).astype(bf),
            "encT": np.ascontiguousarray(encoder_out[b, :T].T).astype(bf),
            "Wq": np.ascontiguousarray(WqF[:, cols]).astype(bf),
            "Wk": np.ascontiguousarray(Wk_all[:, cols]).astype(bf),
            "Wv": np.ascontiguousarray(Wv_all[:, cols]).astype(bf),
            "Wwk": np.ascontiguousarray(Wwk_all[:, cols]).astype(bf),
            "Wwv": np.ascontiguousarray(Wwv_all[:, cols]).astype(bf),
            "WqfA0": np.ascontiguousarray(WqfA0).astype(bf),
            "WqfA1": np.ascontiguousarray(WqfA1).astype(bf),
            "WqfB0": np.ascontiguousarray(WqfB0).astype(bf),
            "WqfB1": np.ascontiguousarray(WqfB1).astype(bf),
            "WkfA": np.ascontiguousarray(WkfA).astype(bf),
            "WkfB": np.ascontiguousarray(WkfB).astype(bf),
            "WoutA": WoutA.astype(bf),
            "mask_pack": mask_pack.astype(bf),
            "ident": ident.astype(bf),
        })
    return in_maps


_BUILD_CACHE = {}


def run_sharded(inputs, T=2048, trace=False):
    if T not in _BUILD_CACHE:
        _BUILD_CACHE[T] = build_program(T=T)
    nc = _BUILD_CACHE[T]
    in_maps = _host_prep(T=T, **inputs)
    res = run_bass_kernel_spmd(nc, in_maps, core_ids=list(range(8)), trace=trace)
    x = inputs["x"]
    B = x.shape[0]
    out = np.array(x[:, :T], np.float32, copy=True)
    for c in range(8):
        out[c // 4] += res.results[c]["out"].T
    return out, res


def kernel(**inputs):
    inputs = {k: np.asarray(v, np.float32) for k, v in inputs.items()}
    out, _ = run_sharded(inputs, T=2048, trace=False)
    return out

